# revision 17
# baseline (speedup 1.0000x reference)
"""Bidirectional Mamba block on 8 Trainium2 NeuronCores.

Sharding: 8 cores = 4 batches x 2 directions (fwd/bwd). Each core runs the
full per-(batch, direction) Mamba pipeline on a time-transposed slice
x[b].T (time-flipped for the backward direction), producing its partial
contribution to the fused output projection. Host sums fwd+bwd partials,
adds the residual and fusion bias.

v2 layout: [d (partitions), t (free)], selective scan restructured:
  - n-loop runs in 2 passes over d-PAIRS using mega tiles [128, 4098]
    (two 2048-column d-tile segments + poisoned boundary columns where
    delta=1e9 -> dA=exp(A*1e9)=0 and u2=0 -> dBu=0, so one
    tensor_tensor_scan instruction covers both segments with a clean
    state reset).
  - B/C broadcast tiles are [128, 2049]; the mega elementwise mults read
    them twice via a stride-0 middle AP dim (keeps DVE 2x mode).
  - y = sum_n h_n*C_n accumulated on the TENSOR engine: per n, 8
    identity matmuls [128x128x512] accumulate yp slices into 8 PSUM
    banks (2 d-tiles x 4 t-chunks). DVE no longer does the adds.
  - gate step 1 fused with the PSUM drain: y2 = u2*D + yacc via
    scalar_tensor_tensor reading PSUM directly.
  - Act ops batched per activation function to minimise table loads.
"""

import numpy as np
import ml_dtypes

import concourse.bass as bass
import concourse.bacc as bacc
import concourse.tile as tile
from concourse import mybir
from concourse.bass_utils import run_bass_kernel_spmd

T = 2048
TP = T + 1          # broadcast tile width (padded)
MW = 2 * T + 2      # mega width: [0:T) seg A, T poison, [T+1:2T+1) seg B, 2T+1 poison
DM = 256      # d_model
DI = 512      # d_inner
DS = 16       # d_state
DR = 16       # dt_rank
NCHUNK = 4    # matmul moving-dim chunks of 512
CH = T // NCHUNK
NDT = DI // 128  # 4 d-tiles of 128 partitions

BF = mybir.dt.bfloat16
F32 = mybir.dt.float32
AF = mybir.ActivationFunctionType
OP = mybir.AluOpType

_CACHE = {}


def _bcast_ap(dram_handle, row, col0, width):
    """AP reading dram[row, col0:col0+width] broadcast across 128 partitions."""
    base = dram_handle[row:row + 1, col0:col0 + width]
    return bass.AP(tensor=base.tensor, offset=base.offset,
                   ap=[[0, 128], [1, width]])


def _rep2_ap(tile_, width):
    """Free-replicated read of tile_[:, 0:width] twice: [128, 2*width]."""
    return bass.AP(tensor=tile_.tensor, offset=tile_.offset,
                   ap=[tile_.ap[0], [0, 2], [1, width]])


def _build(avals):
    nc = bacc.Bacc()

    # --- I/O ---------------------------------------------------------------
    xt = nc.declare_dram_parameter("xt", [DM, T], BF, isOutput=False)
    inwT = nc.declare_dram_parameter("inwT", [DM, 2 * DI], BF, isOutput=False)
    xpwT = nc.declare_dram_parameter("xpwT", [DI, DR + 2 * DS], BF, isOutput=False)
    dtwT = nc.declare_dram_parameter("dtwT", [DR, DI], BF, isOutput=False)
    owT = nc.declare_dram_parameter("owT", [DI, DM], BF, isOutput=False)
    fwT = nc.declare_dram_parameter("fwT", [DM, DM], BF, isOutput=False)
    convw = nc.declare_dram_parameter("convw", [DI, 4], F32, isOutput=False)
    convb = nc.declare_dram_parameter("convb", [DI, 1], F32, isOutput=False)
    dtb = nc.declare_dram_parameter("dtb", [DI, 1], F32, isOutput=False)
    dvec = nc.declare_dram_parameter("dvec", [DI, 1], F32, isOutput=False)
    nw = nc.declare_dram_parameter("nw", [DM, 1], F32, isOutput=False)
    nb = nc.declare_dram_parameter("nb", [DM, 1], F32, isOutput=False)
    idh = nc.declare_dram_parameter("idh", [128, 128], BF, isOutput=False)
    o2 = nc.declare_dram_parameter("o2", [DM, T], F32, isOutput=True)

    # DRAM scratch for partition-broadcast bounces
    stb = nc.dram_tensor("stb", [2, T], BF)         # mean, rstd rows
    bcb = nc.dram_tensor("bcb", [2 * DS, TP], BF)   # B rows 0..15, C rows 16..31

    with tile.TileContext(nc) as tc:
        with (
            tc.tile_pool(name="const", bufs=1) as const,
            tc.tile_pool(name="big", bufs=2) as big,
            tc.tile_pool(name="pers", bufs=4) as pers,
            tc.tile_pool(name="work", bufs=2) as work,
            tc.tile_pool(name="strow", bufs=4) as strow,
        ):
            # --- load x (chunked across DMA queues) -------------------------
            xn = [big.tile([128, T], BF, tag="xn", name="xn", bufs=2)
                  for _ in range(2)]
            for k in range(2):
                for c in range(NCHUNK):
                    cs = slice(c * CH, (c + 1) * CH)
                    nc.sync.dma_start(out=xn[k][:, cs],
                                      in_=xt[k * 128:(k + 1) * 128, cs])

            # --- weights/constants ------------------------------------------
            w_inwT = [const.tile([128, 2 * DI], BF, tag="winw", name="winw",
                                 bufs=2) for _ in range(2)]
            for k in range(2):
                nc.sync.dma_start(out=w_inwT[k], in_=inwT[k * 128:(k + 1) * 128, :])
            w_xpwT = [const.tile([128, DR + 2 * DS], BF, tag="wxpw", name="wxpw",
                                 bufs=NDT) for _ in range(NDT)]
            for k in range(NDT):
                nc.sync.dma_start(out=w_xpwT[k], in_=xpwT[k * 128:(k + 1) * 128, :])
            w_dtwT = const.tile([DR, DI], BF, tag="wdtw", name="wdtw")
            nc.sync.dma_start(out=w_dtwT, in_=dtwT[:, :])
            w_owT = [const.tile([128, DM], BF, tag="wow", name="wow", bufs=NDT)
                     for _ in range(NDT)]
            for k in range(NDT):
                nc.sync.dma_start(out=w_owT[k], in_=owT[k * 128:(k + 1) * 128, :])
            w_fwT = [const.tile([128, DM], BF, tag="wfw", name="wfw", bufs=2)
                     for _ in range(2)]
            for k in range(2):
                nc.sync.dma_start(out=w_fwT[k], in_=fwT[k * 128:(k + 1) * 128, :])
            w_convw = [const.tile([128, 4], F32, tag="wconv", name="wconv",
                                  bufs=NDT) for _ in range(NDT)]
            w_convb = [const.tile([128, 1], F32, tag="wconvb", name="wconvb",
                                  bufs=NDT) for _ in range(NDT)]
            w_dtb = [const.tile([128, 1], F32, tag="wdtb", name="wdtb",
                                bufs=NDT) for _ in range(NDT)]
            w_dvec = [const.tile([128, 1], F32, tag="wdvec", name="wdvec",
                                 bufs=NDT) for _ in range(NDT)]
            for k in range(NDT):
                sl = slice(k * 128, (k + 1) * 128)
                nc.sync.dma_start(out=w_convw[k], in_=convw[sl, :])
                nc.sync.dma_start(out=w_convb[k], in_=convb[sl, :])
                nc.sync.dma_start(out=w_dtb[k], in_=dtb[sl, :])
                nc.sync.dma_start(out=w_dvec[k], in_=dvec[sl, :])
            w_nw = [const.tile([128, 1], F32, tag="wnw", name="wnw", bufs=2)
                    for _ in range(2)]
            w_nb = [const.tile([128, 1], F32, tag="wnb", name="wnb", bufs=2)
                    for _ in range(2)]
            for k in range(2):
                sl = slice(k * 128, (k + 1) * 128)
                nc.sync.dma_start(out=w_nw[k], in_=nw[sl, :])
                nc.sync.dma_start(out=w_nb[k], in_=nb[sl, :])
            ident = const.tile([128, 128], BF, tag="ident", name="ident")
            nc.sync.dma_start(out=ident, in_=idh[:, :])
            ones_bf = const.tile([128, 1], BF, tag="ones", name="ones")
            nc.vector.memset(ones_bf, 1.0)
            eps_t = const.tile([NCHUNK, 1], F32, tag="eps", name="eps")
            nc.vector.memset(eps_t, 1e-5)
            # zero the padded column of the B/C bounce buffer so the
            # broadcast reads a finite value at the mega poison column
            zrow = const.tile([2 * DS, 1], BF, tag="zrow", name="zrow")
            nc.vector.memset(zrow, 0.0)
            nc.sync.dma_start(out=bcb[:, T:TP], in_=zrow)

            # persistent mega tiles (2 d-pairs)
            u2m = [pers.tile([128, MW], BF, tag="u2m", name="u2m", bufs=2)
                   for _ in range(2)]
            dlm = [pers.tile([128, MW], BF, tag="dlm", name="dlm", bufs=2)
                   for _ in range(2)]
            wdm = [pers.tile([128, MW], BF, tag="wdm", name="wdm", bufs=2)
                   for _ in range(2)]
            sz = [pers.tile([128, T], BF, tag="sz", name="sz") for _ in range(NDT)]
            ygc = [pers.tile([128, T], BF, tag="ygc", name="ygc")
                   for _ in range(NDT)]
            # poison columns: delta=1e9, u2=0 at cols T and 2T+1
            for p in range(2):
                nc.vector.memset(dlm[p][:, T:T + 1], 1e9)
                nc.vector.memset(dlm[p][:, MW - 1:MW], 1e9)
                nc.vector.memset(u2m[p][:, T:T + 1], 0.0)
                nc.vector.memset(u2m[p][:, MW - 1:MW], 0.0)

            def mseg(p, k):
                """Segment slice of mega tile for d-tile index (2*p + k)."""
                return slice(k * (T + 1), k * (T + 1) + T)

            with tc.tile_pool(name="ps", bufs=2, space="PSUM") as ps, \
                 tc.tile_pool(name="pss", bufs=2, space="PSUM") as pss:
                # --- LN stats, chunked: mean/rstd rows -> DRAM bounce -------
                for c in range(NCHUNK):
                    cs = slice(c * CH, (c + 1) * CH)
                    pstat_s = pss.tile([1, CH], F32, tag="px", name="pstat_s")
                    pstat_q = pss.tile([1, CH], F32, tag="px", name="pstat_q")
                    for k in range(2):
                        nc.tensor.matmul(pstat_s, ones_bf[:, 0:1], xn[k][:, cs],
                                         start=(k == 0), stop=(k == 1))
                    for k in range(2):
                        xsq_c = work.tile([128, CH], BF, tag="xsq", name="xsq")
                        nc.vector.tensor_mul(xsq_c, xn[k][:, cs], xn[k][:, cs])
                        nc.tensor.matmul(pstat_q, ones_bf[:, 0:1], xsq_c,
                                         start=(k == 0), stop=(k == 1))
                    mean_c = strow.tile([1, CH], F32, tag="st", name="mean_c")
                    nc.scalar.activation(mean_c, pstat_s, AF.Copy, scale=1.0 / DM)
                    msq_c = strow.tile([1, CH], F32, tag="st", name="msq_c")
                    nc.vector.tensor_mul(msq_c, mean_c, mean_c)
                    mean_bf = strow.tile([1, CH], BF, tag="st", name="mean_bf")
                    nc.scalar.copy(out=mean_bf, in_=mean_c)
                    nc.sync.dma_start(out=stb[0:1, cs], in_=mean_bf)
                    var_c = strow.tile([1, CH], F32, tag="st", name="var_c")
                    nc.vector.scalar_tensor_tensor(out=var_c, in0=pstat_q,
                                                   scalar=1.0 / DM, in1=msq_c,
                                                   op0=OP.mult, op1=OP.subtract)
                    srt_c = strow.tile([1, CH], F32, tag="st", name="srt_c")
                    nc.scalar.activation(srt_c, var_c, AF.Sqrt, bias=eps_t[0:1, :],
                                         scale=1.0)
                    rin_c = strow.tile([1, CH], F32, tag="st", name="rin_c")
                    nc.vector.reciprocal(rin_c, srt_c)
                    rstd_c = strow.tile([1, CH], BF, tag="st", name="rstd_c")
                    nc.scalar.copy(out=rstd_c, in_=rin_c)
                    nc.sync.dma_start(out=stb[1:2, cs], in_=rstd_c)

                # broadcast mean/rstd and normalize x in place -> xn (chunked)
                for c in range(NCHUNK):
                    cs = slice(c * CH, (c + 1) * CH)
                    mu_b = big.tile([128, CH], BF, tag="mub", name="mu_b", bufs=2)
                    rs_b = big.tile([128, CH], BF, tag="rsb", name="rs_b", bufs=2)
                    nc.gpsimd.dma_start(out=mu_b, in_=_bcast_ap(stb, 0, c * CH, CH))
                    nc.gpsimd.dma_start(out=rs_b, in_=_bcast_ap(stb, 1, c * CH, CH))
                    for k in range(2):
                        nc.vector.tensor_sub(xn[k][:, cs], xn[k][:, cs], mu_b)
                        nc.vector.tensor_mul(xn[k][:, cs], xn[k][:, cs], rs_b)
                        nc.vector.tensor_scalar(out=xn[k][:, cs], in0=xn[k][:, cs],
                                                scalar1=w_nw[k], scalar2=w_nb[k],
                                                op0=OP.mult, op1=OP.add)

                # --- in-projection u blocks: u -> u2m seg (as raw u), then
                # conv taps read the seg in place, silu overwrites it with u2
                for mb in range(NDT):
                    d = mb
                    p, kk = d // 2, d % 2
                    seg0 = kk * (T + 1)
                    for c in range(NCHUNK):
                        cs = slice(c * CH, (c + 1) * CH)
                        pmm = ps.tile([128, CH], F32, tag="pmm", name="pmm")
                        for k in range(2):
                            nc.tensor.matmul(pmm, w_inwT[k][:, mb * 128:(mb + 1) * 128],
                                             xn[k][:, cs], start=(k == 0), stop=(k == 1))
                        nc.scalar.copy(out=u2m[p][:, seg0 + c * CH:seg0 + (c + 1) * CH],
                                       in_=pmm)
                    useg = u2m[p][:, seg0:seg0 + T]
                    # conv as 4 independent tap products (DVE tensor_scalar,
                    # 4x mode) summed with shifts on the PE into PSUM; silu
                    # reads the PSUM accumulator directly.
                    pcv = pss.tile([128, T], F32, tag="pcv", name="pcv", bufs=1)
                    for k in (3, 2, 1, 0):      # shift s = 3-k; s=0 first
                        s = 3 - k
                        mk = big.tile([128, T], BF, tag="mtap", name="mk", bufs=2)
                        if k == 3:
                            nc.vector.tensor_scalar(out=mk, in0=useg,
                                                    scalar1=w_convw[d][:, 3:4],
                                                    scalar2=w_convb[d],
                                                    op0=OP.mult, op1=OP.add)
                        else:
                            nc.vector.tensor_scalar(out=mk[:, 0:T - s],
                                                    in0=u2m[p][:, seg0:seg0 + T - s],
                                                    scalar1=w_convw[d][:, k:k + 1],
                                                    scalar2=None, op0=OP.mult)
                        for c in range(NCHUNK):
                            lo = max(s, c * CH)
                            hi = (c + 1) * CH
                            nc.tensor.matmul(pcv[:, lo:hi], ident,
                                             mk[:, lo - s:hi - s],
                                             start=(k == 3), stop=(k == 0))
                    nc.scalar.activation(u2m[p][:, seg0:seg0 + T], pcv, AF.Silu,
                                         bias=0.0, scale=1.0)

                # --- x_dbl = xpwT.T @ u2  ([48, T]): dt, B, C ---------------
                dtBC = const.tile([DR + 2 * DS, T], BF, tag="dtbc", name="dtbc")
                for c in range(NCHUNK):
                    cs = slice(c * CH, (c + 1) * CH)
                    pdb = pss.tile([DR + 2 * DS, CH], F32, tag="px", name="pdb")
                    for d in range(NDT):
                        p, kk = d // 2, d % 2
                        seg = slice(kk * (T + 1) + c * CH, kk * (T + 1) + (c + 1) * CH)
                        nc.tensor.matmul(pdb, w_xpwT[d], u2m[p][:, seg],
                                         start=(d == 0), stop=(d == NDT - 1))
                    nc.vector.tensor_scalar(out=dtBC[:, cs], in0=pdb,
                                            scalar1=1.0, scalar2=None,
                                            op0=OP.mult)
                # bounce B and C rows to DRAM for partition-broadcast
                nc.sync.dma_start(out=bcb[:, 0:T], in_=dtBC[DR:DR + 2 * DS, :])

                # --- delta = softplus(dtwT.T @ dt + dt_b) into mega tiles ---
                # wdm is used as Exp scratch before its real fill:
                #   Exp(dtproj + dtb) -> wdm segments; Ln(1 + wdm) -> dlm;
                #   then wdm = dlm * u2m overwrites the scratch.
                # Pair 0 finishes first (its Ln + wdm fill) so pass 0 can
                # start; pair 1's Ln + wdm fill are deferred into the scan
                # region. z-projection blocks run after pair-0's chain.
                def _delta_mm(d):
                    p, kk = d // 2, d % 2
                    for c in range(NCHUNK):
                        seg = slice(kk * (T + 1) + c * CH, kk * (T + 1) + (c + 1) * CH)
                        pda = ps.tile([128, CH], F32, tag="pmm", name="pda")
                        nc.tensor.matmul(pda, w_dtwT[:, d * 128:(d + 1) * 128],
                                         dtBC[0:DR, c * CH:(c + 1) * CH],
                                         start=True, stop=True)
                        nc.scalar.activation(wdm[p][:, seg], pda, AF.Exp,
                                             bias=w_dtb[d], scale=1.0)

                def _delta_fin(p):
                    nc.scalar.activation(dlm[p][:, 0:T], wdm[p][:, 0:T],
                                         AF.Ln, bias=1.0, scale=1.0)
                    nc.scalar.activation(dlm[p][:, T + 1:MW - 1],
                                         wdm[p][:, T + 1:MW - 1],
                                         AF.Ln, bias=1.0, scale=1.0)
                    nc.vector.tensor_mul(wdm[p], dlm[p], u2m[p])

                for d in (0, 1):
                    _delta_mm(d)
                _delta_fin(0)

                # z blocks -> silu -> sz (needed only at drain time)
                for mb in range(NDT, 2 * NDT):
                    for c in range(NCHUNK):
                        cs = slice(c * CH, (c + 1) * CH)
                        pmm = ps.tile([128, CH], F32, tag="pmm", name="pmm")
                        for k in range(2):
                            nc.tensor.matmul(pmm, w_inwT[k][:, mb * 128:(mb + 1) * 128],
                                             xn[k][:, cs], start=(k == 0), stop=(k == 1))
                        nc.scalar.activation(sz[mb - NDT][:, cs], pmm, AF.Silu,
                                             bias=0.0, scale=1.0)
                for d in (2, 3):
                    _delta_mm(d)

            # --- selective scan: 2 passes over d-pairs ----------------------
            with tc.tile_pool(name="psy", bufs=8, space="PSUM") as psy, \
                 tc.tile_pool(name="nb_", bufs=2) as nbp:
                for p in range(2):
                    # 8 psum accumulators: (k in pair, chunk) -> [128, 512]
                    pyac = [psy.tile([128, CH], F32, tag="pyac", name="pyac")
                            for _ in range(8)]
                    for n in range(DS):
                        a_n = float(avals[n])
                        bb = nbp.tile([128, TP], BF, tag="bbn", name="bbn", bufs=2)
                        cb = nbp.tile([128, TP], BF, tag="cbn", name="cbn", bufs=2)
                        nc.gpsimd.dma_start(out=bb, in_=_bcast_ap(bcb, n, 0, TP))
                        nc.gpsimd.dma_start(out=cb, in_=_bcast_ap(bcb, DS + n, 0, TP))
                        dA = nbp.tile([128, MW], BF, tag="dA", name="dA", bufs=2)
                        nc.scalar.activation(dA, dlm[p], AF.Exp, bias=0.0, scale=a_n)
                        dBu = nbp.tile([128, MW], BF, tag="dBu", name="dBu", bufs=1)
                        nc.vector.tensor_tensor(out=dBu, in0=wdm[p],
                                                in1=_rep2_ap(bb, TP), op=OP.mult)
                        h = nbp.tile([128, MW], BF, tag="h", name="h", bufs=2)
                        nc.vector.tensor_tensor_scan(h, dA, dBu, 0.0,
                                                     op0=OP.mult, op1=OP.add)
                        # NOTE: do NOT offload these mults to the Pool engine —
                        # Pool shares SBUF ports with DVE and concurrent Pool
                        # ops slow DVE scans ~1.5x (measured 8.7us -> 12.3us).
                        yp = nbp.tile([128, MW], BF, tag="yp", name="yp", bufs=2)
                        nc.vector.tensor_tensor(out=yp, in0=h,
                                                in1=_rep2_ap(cb, TP), op=OP.mult)
                        for k in range(2):
                            for c in range(NCHUNK):
                                seg = slice(k * (T + 1) + c * CH,
                                            k * (T + 1) + (c + 1) * CH)
                                nc.tensor.matmul(pyac[k * NCHUNK + c], ident,
                                                 yp[:, seg], start=(n == 0),
                                                 stop=(n == DS - 1))
                        if p == 0 and n == 3:
                            _delta_fin(1)   # pair-1 softplus finish + wdm fill
                    # drain + gate: ygc = (u2*D + yacc) * sz
                    for k in range(2):
                        d = 2 * p + k
                        for c in range(NCHUNK):
                            cs = slice(c * CH, (c + 1) * CH)
                            seg = slice(k * (T + 1) + c * CH,
                                        k * (T + 1) + (c + 1) * CH)
                            y2 = work.tile([128, CH], BF, tag="y2c", name="y2c",
                                           bufs=2)
                            nc.vector.scalar_tensor_tensor(
                                out=y2, in0=u2m[p][:, seg], scalar=w_dvec[d],
                                in1=pyac[k * NCHUNK + c], op0=OP.mult, op1=OP.add)
                            nc.vector.tensor_mul(ygc[d][:, cs], y2, sz[d][:, cs])

            # --- epilogue: out-proj + fusion, chunked -----------------------
            with tc.tile_pool(name="pse", bufs=3, space="PSUM") as pse:
                for c in range(NCHUNK):
                    cs = slice(c * CH, (c + 1) * CH)
                    o1c = [work.tile([128, CH], BF, tag="o1c", name="o1c", bufs=4)
                           for _ in range(2)]
                    for mb in range(2):
                        pmo = pse.tile([128, CH], F32, tag="pme", name="pmo")
                        for k in range(NDT):
                            nc.tensor.matmul(pmo, w_owT[k][:, mb * 128:(mb + 1) * 128],
                                             ygc[k][:, cs], start=(k == 0),
                                             stop=(k == NDT - 1))
                        nc.scalar.copy(out=o1c[mb], in_=pmo)
                    for mb in range(2):
                        pmf = pse.tile([128, CH], F32, tag="pme", name="pmf")
                        for k in range(2):
                            nc.tensor.matmul(pmf, w_fwT[k][:, mb * 128:(mb + 1) * 128],
                                             o1c[k], start=(k == 0), stop=(k == 1))
                        osb = work.tile([128, CH], F32, tag="osb", name="osb", bufs=2)
                        nc.scalar.copy(out=osb, in_=pmf)
                        nc.sync.dma_start(out=o2[mb * 128:(mb + 1) * 128, cs], in_=osb)

    nc.finalize()
    return nc


def _prep_core(x_b, inp, pfx, direction, fus_w, norm_w, norm_b, idh):
    """Host-side input map for one core."""
    bf16 = ml_dtypes.bfloat16
    xt = np.ascontiguousarray(x_b.T)
    if direction:
        xt = np.ascontiguousarray(xt[:, ::-1])
    g = lambda k: np.asarray(inp[pfx + k])
    m = {
        "xt": xt.astype(bf16),
        "inwT": np.ascontiguousarray(g("in_w").T).astype(bf16),
        "xpwT": np.ascontiguousarray(g("xproj_w").T).astype(bf16),
        "dtwT": np.ascontiguousarray(g("dt_w").T).astype(bf16),
        "owT": np.ascontiguousarray(g("out_w").T).astype(bf16),
        "fwT": np.ascontiguousarray(
            fus_w[:, direction * DM:(direction + 1) * DM].T).astype(bf16),
        "convw": np.ascontiguousarray(g("conv_w")).astype(np.float32),
        "convb": g("conv_b").reshape(DI, 1).astype(np.float32),
        "dtb": g("dt_b").reshape(DI, 1).astype(np.float32),
        "dvec": g("D").reshape(DI, 1).astype(np.float32),
        "nw": norm_w.reshape(DM, 1).astype(np.float32),
        "nb": norm_b.reshape(DM, 1).astype(np.float32),
        "idh": idh,
    }
    return m


def _run(inputs, trace=False):
    x = np.asarray(inputs["x"], np.float32)
    B = x.shape[0]
    assert x.shape == (4, T, DM), x.shape
    fus_w = np.asarray(inputs["fus_w"], np.float32)
    fus_b = np.asarray(inputs["fus_b"], np.float32)
    norm_w = np.asarray(inputs["norm_w"], np.float32)
    norm_b = np.asarray(inputs["norm_b"], np.float32)
    idh = np.eye(128, dtype=ml_dtypes.bfloat16)

    avals_f = -np.exp(np.asarray(inputs["f_A_log"], np.float32)[0])
    avals_b = -np.exp(np.asarray(inputs["b_A_log"], np.float32)[0])
    assert np.allclose(avals_f, avals_b), "A must match across directions"
    key = avals_f.tobytes()
    if key not in _CACHE:
        _CACHE[key] = _build(avals_f)
    nc = _CACHE[key]

    in_maps = []
    for b in range(B):
        for direction in (0, 1):
            pfx = "b_" if direction else "f_"
            in_maps.append(_prep_core(x[b], inputs, pfx, direction,
                                      fus_w, norm_w, norm_b, idh))

    res = run_bass_kernel_spmd(nc, in_maps, list(range(8)), trace=trace)
    out = np.empty((B, T, DM), np.float32)
    for b in range(B):
        of = res.results[2 * b]["o2"]
        ob = res.results[2 * b + 1]["o2"][:, ::-1]
        out[b] = (of + ob).T + x[b] + fus_b[None, :]
    return out, res


def kernel(**inputs):
    out, _ = _run(inputs, trace=False)
    return out
